# revision 1
# baseline (speedup 1.0000x reference)
# v4: AllGather-based K/V sharing. Each core computes Q/K/V for its own 512
# tokens; K^T and V (with softmax ones-column) are packed into one buffer and
# AllGather'd within each 4-core batch group. Attention then runs against the
# gathered full-sequence K/V in global token order (attention is invariant to
# key order, so rank-major order is fine).
import os
import numpy as np

B, S, D = 2, 2048, 1024
H, DK, DVH, DFF = 16, 64, 64, 4096
TOK = S // 4
NP = H // 2
KTILES = S // 128
KC = D // 128
MH = DFF // 128
EPS = 1e-5
BLK = NP * 512 + 4 * H * 65   # 4096 + 4160 = 8256 cols per partition row

_CACHE = {}


def _build():
    import concourse.mybir as mybir
    import concourse.tile as tile
    from concourse import bacc

    f32, f32r = mybir.dt.float32, mybir.dt.float32r
    Exp = mybir.ActivationFunctionType.Exp
    Sqrt = mybir.ActivationFunctionType.Sqrt
    Ident = mybir.ActivationFunctionType.Identity
    AX = mybir.AxisListType.X
    Alu = mybir.AluOpType

    nc = bacc.Bacc("TRN2", target_bir_lowering=False, debug=False, num_devices=8)

    xb_d = nc.dram_tensor("xb", [TOK, D], f32, kind="ExternalInput")
    wq_d = nc.dram_tensor("wq2", [D, H * DK], f32r, kind="ExternalInput")
    wk_d = nc.dram_tensor("wk2", [D, H * DK], f32r, kind="ExternalInput")
    wv_d = nc.dram_tensor("wv2", [D, H * DVH], f32r, kind="ExternalInput")
    wo_d = nc.dram_tensor("wo", [D, D], f32r, kind="ExternalInput")
    w1_d = nc.dram_tensor("w1", [D, DFF], f32r, kind="ExternalInput")
    w2_d = nc.dram_tensor("w2", [DFF, D], f32r, kind="ExternalInput")
    b1c_d = nc.dram_tensor("b1c", [128, MH], f32, kind="ExternalInput")
    b2r_d = nc.dram_tensor("b2r", [1, D], f32r, kind="ExternalInput")
    g1bc_d = nc.dram_tensor("g1bc", [128, D], f32, kind="ExternalInput")
    h1bc_d = nc.dram_tensor("h1bc", [128, D], f32, kind="ExternalInput")
    g2bc_d = nc.dram_tensor("g2bc", [128, D], f32, kind="ExternalInput")
    h2bc_d = nc.dram_tensor("h2bc", [128, D], f32, kind="ExternalInput")
    ident_d = nc.dram_tensor("ident", [128, 128], f32, kind="ExternalInput")
    ones64_d = nc.dram_tensor("ones64", [1, 64], f32r, kind="ExternalInput")
    ones128_d = nc.dram_tensor("ones128", [1, 128], f32r, kind="ExternalInput")
    onesv_d = nc.dram_tensor("onesv", [128, 64], f32r, kind="ExternalInput")
    y_d = nc.dram_tensor("y_part", [TOK, D], f32, kind="ExternalOutput")

    def ln_apply(pool, t, gbc, hbc, out_ap):
        sums = pool.tile([128, 1], f32, tag="ln_sums", name="ln_sums")
        nc.vector.reduce_sum(sums[:], t[:], axis=AX)
        sq = pool.tile([128, D], f32, tag="ln_sq", name="ln_sq")
        ssq = pool.tile([128, 1], f32, tag="ln_ssq", name="ln_ssq")
        nc.scalar.activation(
            sq[:], t[:], mybir.ActivationFunctionType.Square, accum_out=ssq[:]
        )
        s2 = pool.tile([128, 1], f32, tag="ln_s2", name="ln_s2")
        nc.vector.tensor_mul(s2[:], sums[:], sums[:])
        var0 = pool.tile([128, 1], f32, tag="ln_var0", name="ln_var0")
        nc.vector.tensor_scalar(
            out=var0[:], in0=ssq[:], scalar1=1.0 / D, scalar2=EPS,
            op0=Alu.mult, op1=Alu.add,
        )
        s2b = pool.tile([128, 1], f32, tag="ln_s2b", name="ln_s2b")
        nc.vector.tensor_scalar_mul(s2b[:], s2[:], 1.0 / (D * D))
        var = pool.tile([128, 1], f32, tag="ln_var", name="ln_var")
        nc.vector.tensor_sub(var[:], var0[:], s2b[:])
        sd = pool.tile([128, 1], f32, tag="ln_sd", name="ln_sd")
        nc.scalar.activation(sd[:], var[:], Sqrt)
        rv = pool.tile([128, 1], f32, tag="ln_rv", name="ln_rv")
        nc.vector.reciprocal(rv[:], sd[:])
        nmr = pool.tile([128, 1], f32, tag="ln_nmr", name="ln_nmr")
        nc.vector.tensor_mul(nmr[:], sums[:], rv[:])
        nmr2 = pool.tile([128, 1], f32, tag="ln_nmr2", name="ln_nmr2")
        nc.vector.tensor_scalar_mul(nmr2[:], nmr[:], -1.0 / D)
        xa = pool.tile([128, D], f32, tag="ln_xa", name="ln_xa")
        nc.scalar.activation(xa[:], t[:], Ident, bias=nmr2[:], scale=rv[:])
        xg = pool.tile([128, D], f32, tag="ln_xg", name="ln_xg")
        nc.vector.tensor_mul(xg[:], xa[:], gbc[:])
        nc.vector.tensor_add(out_ap, xg[:], hbc[:])

    with tile.TileContext(nc) as tc:
        with (
            tc.tile_pool(name="const", bufs=1) as cpool,
            tc.tile_pool(name="dram", bufs=1, space="DRAM") as dram,
        ):
            ident = cpool.tile([128, 128], f32)
            nc.sync.dma_start(ident[:], ident_d.ap())
            ones64 = cpool.tile([1, 64], f32r)
            nc.sync.dma_start(ones64[:], ones64_d.ap())
            ones128 = cpool.tile([1, 128], f32r)
            nc.sync.dma_start(ones128[:], ones128_d.ap())
            o_norm = cpool.tile([128, NP, TOK], f32r)

            k_in = dram.tile([128, NP * 512], f32r)
            k_out = dram.tile([8, 128, NP * 512], f32r, addr_space="Shared")
            v_in = dram.tile([128, 4 * H * 65], f32r)
            v_out = dram.tile([8, 128, 4 * H * 65], f32r, addr_space="Shared")

            # ones column of V goes straight into the gather input
            nc.sync.dma_start(
                v_in[:].rearrange("p (s o) -> p s o", o=65)[:, :, 64:65].squeeze(2),
                onesv_d.ap(),
            )

            with tc.tile_pool(name="mid", bufs=1) as midp:
                qT = midp.tile([128, NP, TOK], f32r)

                # ---- Phase A: transpose own x -> xT_own
                with (
                    tc.tile_pool(name="xtp", bufs=1) as xtp,
                    tc.tile_pool(name="pha", bufs=2) as pha,
                    tc.tile_pool(name="ps_tr", bufs=2, space="PSUM") as ps_tr,
                ):
                    xT = xtp.tile([128, KC, TOK], f32r)
                    for tc2 in range(2):
                        xch = pha.tile([128, 2, D], f32, tag="xch", name="xch")
                        nc.sync.dma_start(
                            xch[:],
                            xb_d.ap()[tc2 * 256:(tc2 + 1) * 256, :].rearrange(
                                "(a p) d -> p a d", p=128
                            ),
                        )
                        for dc in range(KC):
                            ps = ps_tr.tile([128, 2, 128], f32, tag="trp", name="trp")
                            for a in range(2):
                                nc.tensor.transpose(
                                    ps[:, a, :],
                                    xch[:, a, dc * 128:(dc + 1) * 128],
                                    ident[:],
                                )
                            nc.vector.tensor_copy(
                                xT[:, dc, tc2 * 256:(tc2 + 1) * 256],
                                ps[:].rearrange("p a t -> p (a t)"),
                            )

                    # ---- Phase B: Q, K, V projections on own tokens
                    with (
                        tc.tile_pool(name="wqk", bufs=3) as wqk,
                        tc.tile_pool(name="stg", bufs=4) as stg,
                        tc.tile_pool(name="ps_q", bufs=2, space="PSUM") as ps_q,
                    ):
                        # K first (feeds the collective)
                        for p in range(NP):
                            wt = wqk.tile([128, KC, 128], f32r, tag="wt", name="wt")
                            nc.sync.dma_start(
                                wt[:],
                                wk_d.ap()[:, p * 128:(p + 1) * 128].rearrange(
                                    "(kc pp) m -> pp kc m", pp=128
                                ),
                            )
                            ps = ps_q.tile([128, 512], f32, tag="psq", name="psq")
                            for kc in range(KC):
                                nc.tensor.matmul(
                                    ps[:], wt[:, kc, :], xT[:, kc, :],
                                    start=(kc == 0), stop=(kc == KC - 1),
                                )
                            st = stg.tile([128, 512], f32r, tag="kst", name="kst")
                            nc.vector.tensor_copy(st[:], ps[:])
                            nc.scalar.dma_start(
                                k_in[:, p * 512:(p + 1) * 512], st[:]
                            )
                        nc.gpsimd.collective_compute(
                            "AllGather",
                            Alu.bypass,
                            ins=[k_in.opt()],
                            outs=[k_out.opt()],
                            replica_groups=[[0, 1, 2, 3, 4, 5, 6, 7]],
                        )
                        # V (own keys)
                        wv_all = wqk.tile([128, KC, H * DVH], f32r, name="wv_all")
                        nc.sync.dma_start(
                            wv_all[:],
                            wv_d.ap().rearrange("(kc p) n -> p kc n", p=128),
                        )
                        for mtk in range(4):
                            for ncc in range(2):
                                ps = ps_q.tile([128, 512], f32, tag="psq", name="psq")
                                for kc in range(KC):
                                    nc.tensor.matmul(
                                        ps[:],
                                        xT[:, kc, mtk * 128:(mtk + 1) * 128],
                                        wv_all[:, kc, ncc * 512:(ncc + 1) * 512],
                                        start=(kc == 0), stop=(kc == KC - 1),
                                    )
                                st = stg.tile([128, 512], f32r, tag="vst", name="vst")
                                nc.vector.tensor_copy(st[:], ps[:])
                                nc.scalar.dma_start(
                                    v_in[:, mtk * 1040 + ncc * 520:]
                                    .rearrange("p (h v) -> p h v", v=65)[:, 0:8, 0:64],
                                    st[:].rearrange("p (h v) -> p h v", h=8),
                                )
                        nc.gpsimd.collective_compute(
                            "AllGather",
                            Alu.bypass,
                            ins=[v_in.opt()],
                            outs=[v_out.opt()],
                            replica_groups=[[0, 1, 2, 3, 4, 5, 6, 7]],
                        )
                        # Q projection (overlaps the collective)
                        for p in range(NP):
                            wt = wqk.tile([128, KC, 128], f32r, tag="wt", name="wt")
                            nc.sync.dma_start(
                                wt[:],
                                wq_d.ap()[:, p * 128:(p + 1) * 128].rearrange(
                                    "(kc pp) m -> pp kc m", pp=128
                                ),
                            )
                            ps = ps_q.tile([128, 512], f32, tag="psq", name="psq")
                            for kc in range(KC):
                                nc.tensor.matmul(
                                    ps[:], wt[:, kc, :], xT[:, kc, :],
                                    start=(kc == 0), stop=(kc == KC - 1),
                                )
                            nc.vector.tensor_copy(qT[:, p, :], ps[:])

                # ---- Phase C: attention against gathered K/V
                with (
                    tc.tile_pool(name="vsb", bufs=1) as vsbp,
                    tc.tile_pool(name="ktp", bufs=3) as ktpool,
                    tc.tile_pool(name="at", bufs=6) as atpool,
                    tc.tile_pool(name="rec", bufs=3) as recpool,
                    tc.tile_pool(name="ps_s", bufs=2, space="PSUM") as ps_s,
                    tc.tile_pool(name="ps_o", bufs=3, space="PSUM") as ps_o,
                    tc.tile_pool(name="ps_r", bufs=1, space="PSUM") as ps_r,
                ):
                    v_sb = vsbp.tile([128, KTILES, H, 65], f32r)
                    pid_sync = nc.sync.partition_id()
                    pid_scal = nc.scalar.partition_id()
                    for r in range(4):
                        for bb in range(2):
                            nc.sync.dma_start(
                                v_sb[:, 4 * r:4 * (r + 1), :, :],
                                v_out[4 * bb + r, :, :].rearrange(
                                    "p (t h v) -> p t h v", t=4, h=H
                                ),
                                cond=(pid_sync < 4) if bb == 0 else (4 <= pid_sync),
                            )
                    for p in range(NP):
                        ktp = ktpool.tile([128, 4, 512], f32r, tag="ktp", name="ktp")
                        for bb in range(2):
                            nc.scalar.dma_start(
                                ktp[:],
                                k_out[4 * bb:4 * (bb + 1), :,
                                      p * 512:(p + 1) * 512].transpose([1, 0, 2]),
                                cond=(pid_scal < 4) if bb == 0 else (4 <= pid_scal),
                            )
                        po = [
                            ps_o.tile([65, TOK], f32, tag="po", name=f"po{p}_{hh}")
                            for hh in range(2)
                        ]
                        for g in range(8):
                            for hh in range(2):
                                sT = ps_s.tile([128, 2, 512], f32, tag="sT", name="sT")
                                for j in range(2):
                                    kt = 2 * g + j
                                    nc.tensor.matmul(
                                        sT[:, j, :],
                                        ktp[hh * 64:(hh + 1) * 64, :, :]
                                        .rearrange("p r t -> p (r t)")[
                                            :, kt * 128:(kt + 1) * 128],
                                        qT[hh * 64:(hh + 1) * 64, p, :],
                                        tile_position=(hh * 64, 0),
                                    )
                                at = atpool.tile([128, 2, 512], f32r, tag="at", name="at")
                                nc.scalar.activation(at[:], sT[:], Exp, scale=0.125)
                                for j in range(2):
                                    kt = 2 * g + j
                                    nc.tensor.matmul(
                                        po[hh][:],
                                        v_sb[:, kt, 2 * p + hh, :],
                                        at[:, j, :],
                                        start=(kt == 0), stop=(kt == KTILES - 1),
                                    )
                        for hh in range(2):
                            rec = recpool.tile([1, TOK], f32r, tag="rec", name="rec")
                            with nc.allow_low_precision(reason="f32r"):
                                nc.vector.reciprocal(rec[:], po[hh][64:65, :])
                            rp = ps_r.tile([64, TOK], f32, tag="rp", name="rp")
                            nc.tensor.matmul(rp[:], ones64[:], rec[:])
                            rsb = recpool.tile([64, TOK], f32, tag="rsb", name="rsb")
                            nc.vector.tensor_copy(rsb[:], rp[:])
                            nc.vector.tensor_mul(
                                o_norm[hh * 64:(hh + 1) * 64, p, :],
                                po[hh][0:64, :],
                                rsb[:],
                            )

            # ---- Phase D: Wo + residual + LN1, then x1 -> x1T
            with tc.tile_pool(name="latex", bufs=1) as latex:
                with (
                    tc.tile_pool(name="wop", bufs=1) as wop,
                    tc.tile_pool(name="dstg", bufs=2) as dstg,
                    tc.tile_pool(name="lnd", bufs=2) as lnd,
                    tc.tile_pool(name="ps_wo", bufs=2, space="PSUM") as ps_wo,
                    tc.tile_pool(name="ps_t2", bufs=2, space="PSUM") as ps_t2,
                ):
                    wo_all = wop.tile([128, KC, D], f32r)
                    for kc in range(KC):
                        nc.sync.dma_start(
                            wo_all[:, kc, :],
                            wo_d.ap()[kc * 128:(kc + 1) * 128, :],
                        )
                    xosb = wop.tile([128, 4, D], f32)
                    nc.scalar.dma_start(
                        xosb[:],
                        xb_d.ap().rearrange("(a p) d -> p a d", p=128),
                    )
                    b2r = latex.tile([1, D], f32r)
                    nc.gpsimd.dma_start(b2r[:], b2r_d.ap())
                    g1bc = latex.tile([128, D], f32)
                    nc.gpsimd.dma_start(g1bc[:], g1bc_d.ap())
                    h1bc = latex.tile([128, D], f32)
                    nc.gpsimd.dma_start(h1bc[:], h1bc_d.ap())
                    g2bc = latex.tile([128, D], f32)
                    nc.gpsimd.dma_start(g2bc[:], g2bc_d.ap())
                    h2bc = latex.tile([128, D], f32)
                    nc.gpsimd.dma_start(h2bc[:], h2bc_d.ap())
                    b1c = latex.tile([128, MH], f32)
                    nc.gpsimd.dma_start(b1c[:], b1c_d.ap())
                    x1 = latex.tile([128, 4, D], f32)
                    x1T = latex.tile([128, KC, TOK], f32r)
                    for mt in range(4):
                        pso = ps_wo.tile([128, 1024], f32, tag="pso", name="pso")
                        for kc in range(KC):
                            for ncc in range(2):
                                nc.tensor.matmul(
                                    pso[:, ncc * 512:(ncc + 1) * 512],
                                    o_norm[:, kc, mt * 128:(mt + 1) * 128],
                                    wo_all[:, kc, ncc * 512:(ncc + 1) * 512],
                                    start=(kc == 0), stop=(kc == KC - 1),
                                )
                        t = dstg.tile([128, D], f32, tag="t1", name="t1")
                        for ncc in range(2):
                            nc.vector.tensor_add(
                                t[:, ncc * 512:(ncc + 1) * 512],
                                pso[:, ncc * 512:(ncc + 1) * 512],
                                xosb[:, mt, ncc * 512:(ncc + 1) * 512],
                            )
                        ln_apply(lnd, t, g1bc, h1bc, x1[:, mt, :])
                    for dc in range(KC):
                        ps = ps_t2.tile([128, 512], f32, tag="trp2", name="trp2")
                        for mt in range(4):
                            nc.tensor.transpose(
                                ps[:, mt * 128:(mt + 1) * 128],
                                x1[:, mt, dc * 128:(dc + 1) * 128],
                                ident[:],
                            )
                            nc.vector.tensor_copy(
                                x1T[:, dc, mt * 128:(mt + 1) * 128],
                                ps[:, mt * 128:(mt + 1) * 128],
                            )

                # ---- Phase E: FFN
                with (
                    tc.tile_pool(name="w1p", bufs=4) as w1p,
                    tc.tile_pool(name="ht", bufs=1) as htp,
                    tc.tile_pool(name="lne", bufs=2) as lne,
                ):
                    hT = htp.tile([128, MH, TOK], f32r)
                    with tc.tile_pool(name="ps_f1", bufs=2, space="PSUM") as ps_f1:
                        for mh in range(MH):
                            w1t = w1p.tile([128, KC, 128], f32r, tag="w1t", name="w1t")
                            (nc.sync if mh % 2 == 0 else nc.scalar).dma_start(
                                w1t[:],
                                w1_d.ap()[:, mh * 128:(mh + 1) * 128].rearrange(
                                    "(kc p) m -> p kc m", p=128
                                ),
                            )
                            ps = ps_f1.tile([128, 512], f32, tag="psf1", name="psf1")
                            for dc in range(KC):
                                nc.tensor.matmul(
                                    ps[:], w1t[:, dc, :], x1T[:, dc, :],
                                    start=(dc == 0), stop=(dc == KC - 1),
                                )
                            nc.vector.tensor_scalar(
                                out=hT[:, mh, :], in0=ps[:],
                                scalar1=b1c[:, mh:mh + 1], scalar2=0.0,
                                op0=Alu.add, op1=Alu.max,
                            )
                    with (
                        tc.tile_pool(name="w2p", bufs=4) as w2p,
                        tc.tile_pool(name="ps_f2", bufs=1, space="PSUM") as ps_f2,
                        tc.tile_pool(name="outp", bufs=2) as outp,
                    ):
                        psy = [
                            [
                                ps_f2.tile([128, 512], f32, tag=f"py{mt}{ncc}",
                                           name=f"py{mt}{ncc}")
                                for ncc in range(2)
                            ]
                            for mt in range(4)
                        ]
                        for mh in range(MH):
                            w2t = w2p.tile([128, D], f32r, tag="w2t", name="w2t")
                            (nc.sync if mh % 2 == 0 else nc.scalar).dma_start(
                                w2t[:], w2_d.ap()[mh * 128:(mh + 1) * 128, :]
                            )
                            for mt in range(4):
                                for ncc in range(2):
                                    nc.tensor.matmul(
                                        psy[mt][ncc][:],
                                        hT[:, mh, mt * 128:(mt + 1) * 128],
                                        w2t[:, ncc * 512:(ncc + 1) * 512],
                                        start=(mh == 0), stop=False,
                                    )
                        for mt in range(4):
                            for ncc in range(2):
                                nc.tensor.matmul(
                                    psy[mt][ncc][:],
                                    ones128[:],
                                    b2r[:, ncc * 512:(ncc + 1) * 512],
                                    start=False, stop=True,
                                )
                        for mt in range(4):
                            t2 = outp.tile([128, D], f32, tag="t2", name="t2")
                            for ncc in range(2):
                                nc.vector.tensor_add(
                                    t2[:, ncc * 512:(ncc + 1) * 512],
                                    psy[mt][ncc][:],
                                    x1[:, mt, ncc * 512:(ncc + 1) * 512],
                                )
                            ot = outp.tile([128, D], f32, tag="ot", name="ot")
                            ln_apply(lne, t2, g2bc, h2bc, ot[:])
                            nc.sync.dma_start(
                                y_d.ap()[mt * 128:(mt + 1) * 128, :], ot[:]
                            )
    nc.compile()
    return nc


def _in_maps(x, Wq, Wk, Wv, Wo, ln1_g, ln1_b, W1, b1, W2, b2, ln2_g, ln2_b):
    x = np.ascontiguousarray(np.asarray(x, np.float32))
    wq2 = np.ascontiguousarray(np.asarray(Wq, np.float32).transpose(1, 0, 2).reshape(D, H * DK))
    wk2 = np.ascontiguousarray(np.asarray(Wk, np.float32).transpose(1, 0, 2).reshape(D, H * DK))
    wv2 = np.ascontiguousarray(np.asarray(Wv, np.float32).transpose(1, 0, 2).reshape(D, H * DVH))
    bcast = lambda v: np.ascontiguousarray(
        np.broadcast_to(np.asarray(v, np.float32), (128, D))
    )
    common = {
        "wq2": wq2, "wk2": wk2, "wv2": wv2,
        "wo": np.ascontiguousarray(np.asarray(Wo, np.float32)),
        "w1": np.ascontiguousarray(np.asarray(W1, np.float32)),
        "w2": np.ascontiguousarray(np.asarray(W2, np.float32)),
        "b1c": np.ascontiguousarray(np.asarray(b1, np.float32).reshape(MH, 128).T),
        "b2r": np.ascontiguousarray(np.asarray(b2, np.float32).reshape(1, D)),
        "g1bc": bcast(ln1_g), "h1bc": bcast(ln1_b),
        "g2bc": bcast(ln2_g), "h2bc": bcast(ln2_b),
        "ident": np.eye(128, dtype=np.float32),
        "ones64": np.ones((1, 64), np.float32),
        "ones128": np.ones((1, 128), np.float32),
        "onesv": np.ones((128, 64), np.float32),
    }
    in_maps = []
    for c in range(8):
        b, q0 = c // 4, TOK * (c % 4)
        m = dict(common)
        m["xb"] = np.ascontiguousarray(x[b, q0:q0 + TOK, :])
        in_maps.append(m)
    return in_maps


def kernel(x, Wq, Wk, Wv, Wo, ln1_g, ln1_b, W1, b1, W2, b2, ln2_g, ln2_b):
    from concourse.bass_utils import run_bass_kernel_spmd

    if "nc" not in _CACHE:
        _CACHE["nc"] = _build()
    nc = _CACHE["nc"]
    in_maps = _in_maps(x, Wq, Wk, Wv, Wo, ln1_g, ln1_b, W1, b1, W2, b2, ln2_g, ln2_b)
    res = run_bass_kernel_spmd(nc, in_maps, core_ids=list(range(8)))
    out = np.empty((B, S, D), np.float32)
    for c in range(8):
        b, q0 = c // 4, TOK * (c % 4)
        out[b, q0:q0 + TOK, :] = res.results[c]["y_part"]
    return out



# revision 8
# speedup vs baseline: 1.5482x; 1.5482x over previous
# v5: single combined K+V AllGather in bf16 over 4-rank replica groups.
# Data-parallel over tokens (8 cores x 512 tokens, 4 cores per batch).
# Each core projects Q/K/V for its own 512 tokens in bf16; K^T and V (with a
# softmax ones-column) are packed into one buffer and AllGather'd within the
# 4-core batch group. Attention runs against the gathered full-sequence K/V.
# All matmul operands are bf16 (PSUM accumulation stays f32); LayerNorm,
# softmax normalization and residuals are f32. Weights are pre-transposed on
# the host into [128, KC, N] layouts so every DMA moves >=1KB contiguous runs.
import numpy as np

B, S, D = 2, 2048, 1024
H, DK, DVH, DFF = 16, 64, 64, 4096
TOK = S // 4
NP = H // 2
KTILES = S // 128
KC = D // 128
MH = DFF // 128
EPS = 1e-5
KCOLS = NP * 512          # 4096 K^T columns per partition row
VCOLS = 4 * H * 65        # 4160 V columns (64 values + 1 ones per head)
BLK = KCOLS + VCOLS       # 8256

_CACHE = {}


def _build():
    import concourse.mybir as mybir
    import concourse.tile as tile
    from concourse import bacc

    f32, f32r, bf16 = mybir.dt.float32, mybir.dt.float32r, mybir.dt.bfloat16
    Exp = mybir.ActivationFunctionType.Exp
    Sqrt = mybir.ActivationFunctionType.Sqrt
    Relu = mybir.ActivationFunctionType.Relu
    Ident = mybir.ActivationFunctionType.Identity
    AX = mybir.AxisListType.X
    Alu = mybir.AluOpType

    nc = bacc.Bacc("TRN2", target_bir_lowering=False, debug=False, num_devices=8)

    xb_d = nc.dram_tensor("xb", [TOK, D], f32, kind="ExternalInput")
    wq_d = nc.dram_tensor("wq_r", [128, KC, H * DK], bf16, kind="ExternalInput")
    wk_d = nc.dram_tensor("wk_r", [128, KC, H * DK], bf16, kind="ExternalInput")
    wv_d = nc.dram_tensor("wv_r", [128, KC, H * DVH], bf16, kind="ExternalInput")
    wo_d = nc.dram_tensor("wo_r", [128, KC, D], bf16, kind="ExternalInput")
    w1_d = nc.dram_tensor("w1_r", [128, KC, DFF], bf16, kind="ExternalInput")
    w2_d = nc.dram_tensor("w2_r", [128, MH, D], bf16, kind="ExternalInput")
    b1c_d = nc.dram_tensor("b1c", [128, MH], f32, kind="ExternalInput")
    b2r_d = nc.dram_tensor("b2r", [1, D], f32r, kind="ExternalInput")
    g1bc_d = nc.dram_tensor("g1bc", [128, D], f32, kind="ExternalInput")
    h1bc_d = nc.dram_tensor("h1bc", [128, D], f32, kind="ExternalInput")
    g2bc_d = nc.dram_tensor("g2bc", [128, D], f32, kind="ExternalInput")
    h2bc_d = nc.dram_tensor("h2bc", [128, D], f32, kind="ExternalInput")
    ident_d = nc.dram_tensor("ident", [128, 128], f32, kind="ExternalInput")
    ones64_d = nc.dram_tensor("ones64", [1, 64], f32r, kind="ExternalInput")
    ones128_d = nc.dram_tensor("ones128", [1, 128], f32r, kind="ExternalInput")
    y_d = nc.dram_tensor("y_part", [TOK, D], f32, kind="ExternalOutput")

    def ln_apply(pool, t, gbc, hbc, out_ap):
        sums = pool.tile([128, 1], f32, tag="ln_sums", name="ln_sums")
        nc.vector.reduce_sum(sums[:], t[:], axis=AX)
        sq = pool.tile([128, D], f32, tag="ln_sq", name="ln_sq")
        ssq = pool.tile([128, 1], f32, tag="ln_ssq", name="ln_ssq")
        nc.scalar.activation(
            sq[:], t[:], mybir.ActivationFunctionType.Square, accum_out=ssq[:]
        )
        s2 = pool.tile([128, 1], f32, tag="ln_s2", name="ln_s2")
        nc.vector.tensor_mul(s2[:], sums[:], sums[:])
        var0 = pool.tile([128, 1], f32, tag="ln_var0", name="ln_var0")
        nc.vector.tensor_scalar(
            out=var0[:], in0=ssq[:], scalar1=1.0 / D, scalar2=EPS,
            op0=Alu.mult, op1=Alu.add,
        )
        s2b = pool.tile([128, 1], f32, tag="ln_s2b", name="ln_s2b")
        nc.vector.tensor_scalar_mul(s2b[:], s2[:], 1.0 / (D * D))
        var = pool.tile([128, 1], f32, tag="ln_var", name="ln_var")
        nc.vector.tensor_sub(var[:], var0[:], s2b[:])
        sd = pool.tile([128, 1], f32, tag="ln_sd", name="ln_sd")
        nc.scalar.activation(sd[:], var[:], Sqrt)
        rv = pool.tile([128, 1], f32, tag="ln_rv", name="ln_rv")
        nc.vector.reciprocal(rv[:], sd[:])
        nmr = pool.tile([128, 1], f32, tag="ln_nmr", name="ln_nmr")
        nc.vector.tensor_mul(nmr[:], sums[:], rv[:])
        nmr2 = pool.tile([128, 1], f32, tag="ln_nmr2", name="ln_nmr2")
        nc.vector.tensor_scalar_mul(nmr2[:], nmr[:], -1.0 / D)
        xa = pool.tile([128, D], f32, tag="ln_xa", name="ln_xa")
        nc.scalar.activation(xa[:], t[:], Ident, bias=nmr2[:], scale=rv[:])
        xg = pool.tile([128, D], f32, tag="ln_xg", name="ln_xg")
        nc.vector.tensor_mul(xg[:], xa[:], gbc[:])
        nc.vector.tensor_add(out_ap, xg[:], hbc[:])

    with tile.TileContext(nc) as tc:
        with (
            tc.tile_pool(name="const", bufs=1) as cpool,
            tc.tile_pool(name="dram", bufs=1, space="DRAM") as dram,
        ):
            ident = cpool.tile([128, 128], f32)
            nc.sync.dma_start(ident[:], ident_d.ap())
            ones64 = cpool.tile([1, 64], f32r)
            nc.sync.dma_start(ones64[:], ones64_d.ap())
            ones128 = cpool.tile([1, 128], f32r)
            nc.sync.dma_start(ones128[:], ones128_d.ap())
            b2r = cpool.tile([1, D], f32r)
            nc.gpsimd.dma_start(b2r[:], b2r_d.ap())
            g1bc = cpool.tile([128, D], f32)
            nc.gpsimd.dma_start(g1bc[:], g1bc_d.ap())
            h1bc = cpool.tile([128, D], f32)
            nc.gpsimd.dma_start(h1bc[:], h1bc_d.ap())
            g2bc = cpool.tile([128, D], f32)
            nc.gpsimd.dma_start(g2bc[:], g2bc_d.ap())
            h2bc = cpool.tile([128, D], f32)
            nc.gpsimd.dma_start(h2bc[:], h2bc_d.ap())
            b1c = cpool.tile([128, MH], f32)
            nc.gpsimd.dma_start(b1c[:], b1c_d.ap())
            # own x, kept resident for transpose source + attention residual
            xfull = cpool.tile([128, 4, D], f32)
            nc.sync.dma_start(
                xfull[:], xb_d.ap().rearrange("(a p) d -> p a d", p=128)
            )
            o_norm = cpool.tile([128, NP, TOK], bf16)

            comb_in = dram.tile([128, BLK], bf16)
            comb_out = dram.tile([4, 128, BLK], bf16)

            with (
                tc.tile_pool(name="mid", bufs=1) as midp,
                tc.tile_pool(name="qp", bufs=1) as qp,
            ):
                qT = qp.tile([128, NP, TOK], bf16)
                x1 = midp.tile([128, 4, D], f32)
                x1T = midp.tile([128, KC, TOK], bf16)

                # ---- Phase A: transpose own x -> xT (bf16)
                with tc.tile_pool(name="xtp", bufs=1) as xtp:
                    xT = xtp.tile([128, KC, TOK], bf16)
                    with tc.tile_pool(name="ps_tr", bufs=2, space="PSUM") as ps_tr:
                        for dc in range(KC):
                            ps = ps_tr.tile([128, 4, 128], f32, tag="trp", name="trp")
                            for a in range(4):
                                nc.tensor.transpose(
                                    ps[:, a, :],
                                    xfull[:, a, dc * 128:(dc + 1) * 128],
                                    ident[:],
                                )
                            nc.vector.tensor_copy(
                                xT[:, dc, :], ps[:].rearrange("p a t -> p (a t)")
                            )

                    # ---- Phase B: Q, K, V projections on own tokens (bf16)
                    with (
                        tc.tile_pool(name="wqk", bufs=1) as wqk,
                        tc.tile_pool(name="stg", bufs=4) as stg,
                        tc.tile_pool(name="ps_q", bufs=2, space="PSUM") as ps_q,
                    ):
                        wk_sb = wqk.tile([128, KC, H * DK], bf16, name="wk_sb")
                        nc.sync.dma_start(wk_sb[:], wk_d.ap())
                        wv_sb = wqk.tile([128, KC, H * DVH], bf16, name="wv_sb")
                        nc.scalar.dma_start(wv_sb[:], wv_d.ap())
                        wq_sb = wqk.tile([128, KC, H * DK], bf16, name="wq_sb")
                        nc.sync.dma_start(wq_sb[:], wq_d.ap())

                        # K first (feeds the collective)
                        for p in range(NP):
                            ps = ps_q.tile([128, 512], f32, tag="psq", name="psq")
                            for kc in range(KC):
                                nc.tensor.matmul(
                                    ps[:],
                                    wk_sb[:, kc, p * 128:(p + 1) * 128],
                                    xT[:, kc, :],
                                    start=(kc == 0), stop=(kc == KC - 1),
                                )
                            st = stg.tile([128, 512], bf16, tag="kst", name="kst")
                            if p % 2 == 0:
                                nc.vector.tensor_copy(st[:], ps[:])
                            else:
                                nc.scalar.activation(st[:], ps[:], Ident)
                            (nc.sync if p % 2 == 0 else nc.scalar).dma_start(
                                comb_in[:, p * 512:(p + 1) * 512], st[:]
                            )
                        # V with interleaved softmax-ones column
                        vstg = wqk.tile([128, 4, H, 65], bf16, name="vstg")
                        nc.gpsimd.memset(vstg[:, :, :, 64:65], 1.0)
                        for mtk in range(4):
                            for ncc in range(2):
                                ps = ps_q.tile([128, 512], f32, tag="psq", name="psq")
                                for kc in range(KC):
                                    nc.tensor.matmul(
                                        ps[:],
                                        xT[:, kc, mtk * 128:(mtk + 1) * 128],
                                        wv_sb[:, kc, ncc * 512:(ncc + 1) * 512],
                                        start=(kc == 0), stop=(kc == KC - 1),
                                    )
                                if ncc == 0:
                                    nc.vector.tensor_copy(
                                        vstg[:, mtk, 0:8, 0:64],
                                        ps[:].rearrange("p (h v) -> p h v", h=8),
                                    )
                                else:
                                    nc.scalar.activation(
                                        vstg[:, mtk, 8:16, 0:64],
                                        ps[:].rearrange("p (h v) -> p h v", h=8),
                                        Ident,
                                    )
                        nc.sync.dma_start(
                            comb_in[:, KCOLS:].rearrange(
                                "p (t h v) -> p t h v", t=4, h=H
                            ),
                            vstg[:],
                        )
                        nc.gpsimd.collective_compute(
                            "AllGather",
                            Alu.bypass,
                            ins=[comb_in.opt()],
                            outs=[comb_out.opt()],
                            replica_groups=[[0, 1, 2, 3], [4, 5, 6, 7]],
                        )
                        # Q projection + weight prefetch overlap the collective
                        for p in range(NP):
                            ps = ps_q.tile([128, 512], f32, tag="psq", name="psq")
                            for kc in range(KC):
                                nc.tensor.matmul(
                                    ps[:],
                                    wq_sb[:, kc, p * 128:(p + 1) * 128],
                                    xT[:, kc, :],
                                    start=(kc == 0), stop=(kc == KC - 1),
                                )
                            if p % 2 == 0:
                                nc.vector.tensor_copy(qT[:, p, :], ps[:])
                            else:
                                nc.scalar.activation(qT[:, p, :], ps[:], Ident)

                # ---- Phase C: attention against gathered K/V
                wop_cm = tc.tile_pool(name="wop", bufs=1)
                wop = wop_cm.__enter__()
                wosb = wop.tile([128, KC, D], bf16)
                nc.gpsimd.dma_start(wosb[:], wo_d.ap())
                with (
                    tc.tile_pool(name="vsb", bufs=1) as vsbp,
                    tc.tile_pool(name="ktp", bufs=3) as ktpool,
                    tc.tile_pool(name="at", bufs=6) as atpool,
                    tc.tile_pool(name="rec", bufs=3) as recpool,
                    tc.tile_pool(name="ps_s", bufs=2, space="PSUM") as ps_s,
                    tc.tile_pool(name="ps_o", bufs=3, space="PSUM") as ps_o,
                    tc.tile_pool(name="ps_r", bufs=1, space="PSUM") as ps_r,
                ):
                    v_sb = vsbp.tile([128, KTILES, H, 65], bf16)
                    for r in range(4):
                        (nc.sync if r % 2 == 0 else nc.scalar).dma_start(
                            v_sb[:, 4 * r:4 * (r + 1), :, :],
                            comb_out[r, :, KCOLS:].rearrange(
                                "p (t h v) -> p t h v", t=4, h=H
                            ),
                        )
                    for p in range(NP):
                        ktp = ktpool.tile([128, 4, 512], bf16, tag="ktp", name="ktp")
                        (nc.sync if p % 2 == 0 else nc.scalar).dma_start(
                            ktp[:],
                            comb_out[0:4, :, p * 512:(p + 1) * 512].transpose(
                                [1, 0, 2]
                            ),
                        )
                        po = [
                            ps_o.tile([65, TOK], f32, tag="po", name=f"po{p}_{hh}")
                            for hh in range(2)
                        ]
                        for g in range(8):
                            for hh in range(2):
                                sT = ps_s.tile([128, 2, 512], f32, tag="sT", name="sT")
                                for j in range(2):
                                    kt = 2 * g + j
                                    nc.tensor.matmul(
                                        sT[:, j, :],
                                        ktp[hh * 64:(hh + 1) * 64, :, :]
                                        .rearrange("p r t -> p (r t)")[
                                            :, kt * 128:(kt + 1) * 128],
                                        qT[hh * 64:(hh + 1) * 64, p, :],
                                        tile_position=(hh * 64, 0),
                                    )
                                at = atpool.tile([128, 2, 512], bf16, tag="at", name="at")
                                nc.scalar.activation(at[:], sT[:], Exp, scale=0.125)
                                for j in range(2):
                                    kt = 2 * g + j
                                    nc.tensor.matmul(
                                        po[hh][:],
                                        v_sb[:, kt, 2 * p + hh, :],
                                        at[:, j, :],
                                        start=(kt == 0), stop=(kt == KTILES - 1),
                                    )
                        for hh in range(2):
                            rec = recpool.tile([1, TOK], f32r, tag="rec", name="rec")
                            with nc.allow_low_precision(reason="f32r"):
                                nc.vector.reciprocal(rec[:], po[hh][64:65, :])
                            rp = ps_r.tile([64, TOK], f32, tag="rp", name="rp")
                            nc.tensor.matmul(rp[:], ones64[:], rec[:])
                            rsb = recpool.tile([64, TOK], f32, tag="rsb", name="rsb")
                            nc.vector.tensor_copy(rsb[:], rp[:])
                            nc.vector.tensor_mul(
                                o_norm[hh * 64:(hh + 1) * 64, p, :],
                                po[hh][0:64, :],
                                rsb[:],
                            )

                # ---- Phase D: Wo + residual + LN1, then x1 -> x1T (bf16)
                with (
                    tc.tile_pool(name="dstg", bufs=2) as dstg,
                    tc.tile_pool(name="lnd", bufs=2) as lnd,
                    tc.tile_pool(name="ps_wo", bufs=2, space="PSUM") as ps_wo,
                    tc.tile_pool(name="ps_t2", bufs=2, space="PSUM") as ps_t2,
                ):
                    for mt in range(4):
                        pso = ps_wo.tile([128, D], f32, tag="pso", name="pso")
                        for kc in range(KC):
                            for ncc in range(2):
                                nc.tensor.matmul(
                                    pso[:, ncc * 512:(ncc + 1) * 512],
                                    o_norm[:, kc, mt * 128:(mt + 1) * 128],
                                    wosb[:, kc, ncc * 512:(ncc + 1) * 512],
                                    start=(kc == 0), stop=(kc == KC - 1),
                                )
                        t = dstg.tile([128, D], f32, tag="t1", name="t1")
                        nc.vector.tensor_add(t[:], pso[:], xfull[:, mt, :])
                        ln_apply(lnd, t, g1bc, h1bc, x1[:, mt, :])
                    for dc in range(KC):
                        ps = ps_t2.tile([128, 512], f32, tag="trp2", name="trp2")
                        for mt in range(4):
                            nc.tensor.transpose(
                                ps[:, mt * 128:(mt + 1) * 128],
                                x1[:, mt, dc * 128:(dc + 1) * 128],
                                ident[:],
                            )
                        nc.vector.tensor_copy(x1T[:, dc, :], ps[:])

                wop_cm.__exit__(None, None, None)

                # ---- Phase E: FFN
                with (
                    tc.tile_pool(name="ht", bufs=1) as htp,
                    tc.tile_pool(name="lne", bufs=2) as lne,
                ):
                    hT = htp.tile([128, MH, TOK], bf16)
                    with (
                        tc.tile_pool(name="w1p", bufs=4) as w1p,
                        tc.tile_pool(name="ps_f1", bufs=2, space="PSUM") as ps_f1,
                    ):
                        w1t = None
                        for mh in range(MH):
                            if mh % 2 == 0:
                                w1t = w1p.tile([128, KC, 256], bf16, tag="w1t",
                                               name="w1t")
                                (nc.sync if mh % 4 == 0 else nc.scalar).dma_start(
                                    w1t[:],
                                    w1_d.ap()[:, :, mh * 128:(mh + 2) * 128],
                                )
                            ps = ps_f1.tile([128, 512], f32, tag="psf1", name="psf1")
                            for dc in range(KC):
                                nc.tensor.matmul(
                                    ps[:],
                                    w1t[:, dc, (mh % 2) * 128:(mh % 2 + 1) * 128],
                                    x1T[:, dc, :],
                                    start=(dc == 0), stop=(dc == KC - 1),
                                )
                            if mh % 2 == 0:
                                nc.scalar.activation(
                                    hT[:, mh, :], ps[:], Relu,
                                    bias=b1c[:, mh:mh + 1],
                                )
                            else:
                                nc.vector.tensor_scalar(
                                    out=hT[:, mh, :], in0=ps[:],
                                    scalar1=b1c[:, mh:mh + 1], scalar2=0.0,
                                    op0=Alu.add, op1=Alu.max,
                                )
                    with (
                        tc.tile_pool(name="w2p", bufs=4) as w2p,
                        tc.tile_pool(name="ps_f2", bufs=1, space="PSUM") as ps_f2,
                        tc.tile_pool(name="outp", bufs=2) as outp,
                    ):
                        psy = [
                            ps_f2.tile([128, D], f32, tag=f"py{mt}", name=f"py{mt}")
                            for mt in range(4)
                        ]
                        for mh in range(MH):
                            w2t = w2p.tile([128, D], bf16, tag="w2t", name="w2t")
                            (nc.sync if mh % 2 == 0 else nc.scalar).dma_start(
                                w2t[:], w2_d.ap()[:, mh, :]
                            )
                            for mt in range(4):
                                for ncc in range(2):
                                    nc.tensor.matmul(
                                        psy[mt][:, ncc * 512:(ncc + 1) * 512],
                                        hT[:, mh, mt * 128:(mt + 1) * 128],
                                        w2t[:, ncc * 512:(ncc + 1) * 512],
                                        start=(mh == 0), stop=False,
                                    )
                        for mt in range(4):
                            for ncc in range(2):
                                nc.tensor.matmul(
                                    psy[mt][:, ncc * 512:(ncc + 1) * 512],
                                    ones128[:],
                                    b2r[:, ncc * 512:(ncc + 1) * 512],
                                    start=False, stop=(ncc == 1),
                                )
                        for mt in range(4):
                            t2 = outp.tile([128, D], f32, tag="t2", name="t2")
                            nc.vector.tensor_add(t2[:], psy[mt][:], x1[:, mt, :])
                            ot = outp.tile([128, D], f32, tag="ot", name="ot")
                            ln_apply(lne, t2, g2bc, h2bc, ot[:])
                            nc.sync.dma_start(
                                y_d.ap()[mt * 128:(mt + 1) * 128, :], ot[:]
                            )
    nc.compile()
    return nc


def _in_maps(x, Wq, Wk, Wv, Wo, ln1_g, ln1_b, W1, b1, W2, b2, ln2_g, ln2_b):
    import ml_dtypes

    bf16 = ml_dtypes.bfloat16
    x = np.ascontiguousarray(np.asarray(x, np.float32))

    def to_sb(w, ncols):
        # [D_in, N] -> [128, D_in//128, N] partition-major layout, bf16
        w = np.asarray(w, np.float32).reshape(-1, 128, ncols).transpose(1, 0, 2)
        return np.ascontiguousarray(w.astype(bf16))

    wq2 = np.asarray(Wq, np.float32).transpose(1, 0, 2).reshape(D, H * DK)
    wk2 = np.asarray(Wk, np.float32).transpose(1, 0, 2).reshape(D, H * DK)
    wv2 = np.asarray(Wv, np.float32).transpose(1, 0, 2).reshape(D, H * DVH)
    bcast = lambda v: np.ascontiguousarray(
        np.broadcast_to(np.asarray(v, np.float32), (128, D))
    )
    common = {
        "wq_r": to_sb(wq2, H * DK), "wk_r": to_sb(wk2, H * DK),
        "wv_r": to_sb(wv2, H * DVH),
        "wo_r": to_sb(np.asarray(Wo, np.float32), D),
        "w1_r": to_sb(np.asarray(W1, np.float32), DFF),
        "w2_r": to_sb(np.asarray(W2, np.float32), D),
        "b1c": np.ascontiguousarray(np.asarray(b1, np.float32).reshape(MH, 128).T),
        "b2r": np.ascontiguousarray(np.asarray(b2, np.float32).reshape(1, D)),
        "g1bc": bcast(ln1_g), "h1bc": bcast(ln1_b),
        "g2bc": bcast(ln2_g), "h2bc": bcast(ln2_b),
        "ident": np.eye(128, dtype=np.float32),
        "ones64": np.ones((1, 64), np.float32),
        "ones128": np.ones((1, 128), np.float32),
    }
    in_maps = []
    for c in range(8):
        b, q0 = c // 4, TOK * (c % 4)
        m = dict(common)
        m["xb"] = np.ascontiguousarray(x[b, q0:q0 + TOK, :])
        in_maps.append(m)
    return in_maps


def kernel(x, Wq, Wk, Wv, Wo, ln1_g, ln1_b, W1, b1, W2, b2, ln2_g, ln2_b):
    from concourse.bass_utils import run_bass_kernel_spmd

    if "nc" not in _CACHE:
        _CACHE["nc"] = _build()
    nc = _CACHE["nc"]
    in_maps = _in_maps(x, Wq, Wk, Wv, Wo, ln1_g, ln1_b, W1, b1, W2, b2, ln2_g, ln2_b)
    res = run_bass_kernel_spmd(nc, in_maps, core_ids=list(range(8)))
    out = np.empty((B, S, D), np.float32)
    for c in range(8):
        b, q0 = c // 4, TOK * (c % 4)
        out[b, q0:q0 + TOK, :] = res.results[c]["y_part"]
    return out
